# revision 9
# baseline (speedup 1.0000x reference)
"""BiAttention kernel for Trainium2 (8 NeuronCores, data-parallel over batch).

Computation (per batch b):
  energy[s, h] = tanh( enc[s, :] @ W_e.T + (hidden[b] @ W_h.T + attn_b) )
  att[s]       = energy[s, :] @ v
  out[b, s]    = softmax(att)[s]

Device strategy (per core, 2 batches each):
  - Host pre-transposes encoder_outputs to k-major [B, 2H, S] so the device
    streams it straight into the PE array as the matmul moving operand.
  - energy^T computed as [h=128 partitions, tokens] so the (hidden@W_h + b)
    term folds into the Tanh activation's per-partition bias.
  - v-reduction: the two h-chunks are folded on DVE first via one fused
    scalar_tensor_tensor (w = t_a * (va/vb) + t_b, with channel pairs
    host-swapped so va <= vb), then ONE matmul per subtile with vb in the
    stationary column finishes the partition reduce. This cuts PE work
    from 10 to 9 moving passes per token vs. the two-matmul v-reduce.
  - reduce rows land directly in shared psum tiles (2 rows per bank at
    partitions 0/64 via the stationary column parity); exp reads psum
    straight, with a per-partition bias that zeroes the pad rows' softmax
    contribution. Batch totals via tiny PE matmuls; the final batch's
    scale is split across DVE and ACT and stored with two parallel DMAs.
"""

import os
import sys
import numpy as np
from contextlib import ExitStack

if "/opt/trn_rl_repo" not in sys.path:
    sys.path.insert(0, "/opt/trn_rl_repo")

from concourse import bass, bacc, tile, mybir
from concourse.bass_utils import run_bass_kernel_spmd

B, S, H = 16, 8192, 256
NCORES = 8
BPC = B // NCORES          # batches per core
GT = int(os.environ.get("K_GT", "1024"))  # tokens per DMA group
ST = 512                   # tokens per compute subtile / psum bank
NSI = GT // ST
NG = S // GT               # DMA groups per batch
NR = S // ST               # rows in the per-batch attention tile (16)
NKC = 4                    # k chunks (2H=512 -> 4x128)
NCB = 8                    # att column blocks (2 rows each)
NHC = 2                    # h chunks (H=256 -> 2x128)

F32 = mybir.dt.float32
F32R = mybir.dt.float32r
BF16 = mybir.dt.bfloat16
NPBF16 = mybir.dt.np(BF16)
AF = mybir.ActivationFunctionType
ALU = mybir.AluOpType
AX = mybir.AxisListType

_CACHE = {}

LAST_RESULT = None
LAST_IN_MAPS = None


def _build(reps=1):
    key = ("nc", reps)
    if key in _CACHE:
        return _CACHE[key]

    nc = bacc.Bacc("TRN2", target_bir_lowering=False, debug=False,
                   num_devices=NCORES)

    encT_d = nc.dram_tensor("encT", [BPC, NKC, 128, S], BF16, kind="ExternalInput").ap()
    wT_d = nc.dram_tensor("wT", [NKC, 128, H], BF16, kind="ExternalInput").ap()
    biasT_d = nc.dram_tensor("biasT", [BPC, NHC, 128, 1], F32, kind="ExternalInput").ap()
    rT_d = nc.dram_tensor("rT", [128, 1], F32, kind="ExternalInput").ap()
    vbT_d = nc.dram_tensor("vbT", [128, 1], BF16, kind="ExternalInput").ap()
    out_d = nc.dram_tensor("out", [BPC, S], F32, kind="ExternalOutput").ap()

    split_first = os.environ.get("K_SPLITFIRST", "0") == "1"

    with tile.TileContext(nc) as tc, ExitStack() as ctx:
        wpool = ctx.enter_context(tc.tile_pool(name="wpool", bufs=1))
        cpool = ctx.enter_context(tc.tile_pool(name="cpool", bufs=1))
        enc_pool = ctx.enter_context(tc.tile_pool(
            name="enc", bufs=int(os.environ.get("K_ENCBUFS", "8"))))
        tanh_pool = ctx.enter_context(tc.tile_pool(name="tanh", bufs=int(os.environ.get("K_TANH", "8"))))
        fold_pool = ctx.enter_context(tc.tile_pool(name="fold", bufs=int(os.environ.get("K_FOLD", "6"))))
        stat_pool = ctx.enter_context(tc.tile_pool(name="stat", bufs=4))
        out_pool = ctx.enter_context(tc.tile_pool(name="outp", bufs=int(os.environ.get("K_OUTP", "2"))))
        epsum_pool = ctx.enter_context(tc.tile_pool(
            name="epsum", bufs=int(os.environ.get("K_EPSUM", "5")), space="PSUM"))
        apsum_pool = ctx.enter_context(tc.tile_pool(
            name="apsum", bufs=int(os.environ.get("K_APSUM", "1")), space="PSUM"))
        cb_pool = ctx.enter_context(tc.tile_pool(
            name="cbp", bufs=int(os.environ.get("K_CB", "2")), space="PSUM"))

        # --- preamble: w[0] first so the first matmul can start, then the
        # first enc group, then the remaining weights ---
        w_all = wpool.tile([128, NKC, H], BF16, tag="w_all")
        w_sb = [w_all[:, kc, :] for kc in range(NKC)]
        chunks0 = []
        if split_first:
            # w0, then the first 512 tokens of all four k-chunks, then the
            # remaining weights, then the second halves: the first subtile's
            # matmul chain starts as soon as the four halves land
            nc.sync.dma_start(w_all[:, 0, :], wT_d[0])
            for kc in range(NKC):
                c = enc_pool.tile([128, GT], BF16, tag="enc", name=f"c0_{kc}")
                nc.sync.dma_start(c[:, 0:ST], encT_d[0, kc, :, 0:ST])
                chunks0.append(c)
            if os.environ.get("K_WSPLIT", "1") == "1":
                for kc in range(1, NKC):
                    nc.sync.dma_start(w_all[:, kc, :], wT_d[kc])
            else:
                nc.sync.dma_start(w_all[:, 1:, :],
                                  wT_d[1:].rearrange("kc p h -> p kc h"))
            for kc in range(NKC):
                nc.sync.dma_start(chunks0[kc][:, ST:GT],
                                  encT_d[0, kc, :, ST:GT])
        else:
            # group 0 as four single-chunk DMAs (earliest possible first
            # matmul); later groups use kc-pair DMAs for fewer sem waits
            if os.environ.get("K_C0FIRST", "0") != "1":
                nc.sync.dma_start(w_all[:, 0, :], wT_d[0])
            if os.environ.get("K_WILV", "1") == "1":
                # w0 c0 c1 w1 c2 w2 c3 w3: each w_kc queued right after the
                # c chunk that precedes its matmul's need
                for kc in range(NKC):
                    c = enc_pool.tile([128, GT], BF16, tag="enc",
                                      name=f"c0_{kc}")
                    nc.sync.dma_start(c[:], encT_d[0, kc, :, 0:GT])
                    chunks0.append(c)
                    if kc >= 1 or os.environ.get("K_C0FIRST", "0") == "1":
                        nc.sync.dma_start(w_all[:, kc, :], wT_d[kc])
            else:
                for kc in range(NKC):
                    c = enc_pool.tile([128, GT], BF16, tag="enc",
                                      name=f"c0_{kc}")
                    nc.sync.dma_start(c[:], encT_d[0, kc, :, 0:GT])
                    chunks0.append(c)
                if os.environ.get("K_WSPLIT", "1") == "1":
                    for kc in range(1, NKC):
                        nc.sync.dma_start(w_all[:, kc, :], wT_d[kc])
                else:
                    nc.sync.dma_start(w_all[:, 1:, :],
                                      wT_d[1:].rearrange("kc p h -> p kc h"))
        bias_all = wpool.tile([128, BPC * NHC], F32, tag="bias_all")
        nc.gpsimd.dma_start(bias_all[:],
                            biasT_d.rearrange("b hc p x -> p (b hc x)"))
        bias_sb = [[bias_all[:, b * NHC + hc:b * NHC + hc + 1]
                    for hc in range(NHC)] for b in range(BPC)]
        # reduce stationary: vb (the larger v of each host-paired channel
        # pair) sits in column 0 for even rows and column 64 for odd rows,
        # zeros elsewhere. Each reduce matmul writes the full 128-row psum
        # bank, real row at partition 0 or 64, zeros everywhere else. The
        # smaller v of each pair is pre-folded on DVE via r = va/vb.
        s_all = wpool.tile([128, 2, 128], BF16, tag="s_all")
        nc.gpsimd.memset(s_all[:], 0.0)
        nc.gpsimd.dma_start(s_all[:, 0, 0:1], vbT_d)
        nc.gpsimd.dma_start(s_all[:, 1, 64:65], vbT_d)
        s_sb = [s_all[:, par, :] for par in range(2)]
        r_col = wpool.tile([128, 1], F32, tag="r_col")
        nc.gpsimd.dma_start(r_col[:], rT_d)

        ones4 = cpool.tile([1, 4], F32, tag="ones4")
        nc.gpsimd.memset(ones4[:], 1.0)
        # dependency-free warmup activation so the 1283ns ACT function-table
        # load happens at t~0 instead of lazily before the first tanh
        warm = cpool.tile([1, 1], F32, tag="warm")
        nc.gpsimd.memset(warm[:], 0.0)
        nc.scalar.activation(warm[:], warm[:], AF.Tanh)
        ones128 = cpool.tile([1, 128], F32, tag="ones128")
        nc.gpsimd.memset(ones128[:], 1.0)
        onescol = cpool.tile([128, 1], F32, tag="onescol")
        nc.gpsimd.memset(onescol[:], 1.0)
        # Constant softmax shift: out = exp(att - 40) / sum(exp(att - 40)).
        # Shift-invariant exactly; |att| <= sum|v| <= 128 and exp(128-40)
        # stays finite in fp32, so no overflow for any input to this model.
        # exp bias: -40 on the two real att rows (0, 64), -200 elsewhere so
        # the padded zero rows contribute exp(-200) == 0 to the softmax total
        # (at exp(0-40) the 126*4096 pad rows add ~2.2e-12, up to 10% of Z)
        cneg = cpool.tile([128, 1], F32, tag="cneg")
        nc.gpsimd.memset(cneg[:], -200.0)
        nc.gpsimd.memset(cneg[0:1, :], -40.0)
        nc.gpsimd.memset(cneg[64:65, :], -40.0)

        def v_reduce(w, cbt, r):
            # att[r, :] = sum_p vb[p] * w[p, :] -- one accumulating
            # full-bank matmul per row with vb in stationary column r%2*64;
            # the [128, ST] psum group spans both rows of column block r//2
            # (start on the even row, stop on the odd row), leaving row
            # r%2*64 real and all other rows zero.
            par = r % 2
            nc.tensor.matmul(cbt[:], s_sb[par], w[:],
                             start=(par == 0), stop=(par == 1))

        def emit_exp_cb(cbt, exp_sb, sums4, cb):
            # exp of column block cb straight from its fully-written psum
            # tile (rows 0/64 real, rest exp(-40) noise ~1e-18, negligible
            # in the total); partial sums land in sums4[:, cb]
            nc.scalar.activation(exp_sb[:, cb * ST:(cb + 1) * ST],
                                 cbt[:], AF.Exp, bias=cneg[:],
                                 accum_out=sums4[:, cb:cb + 1])

        def emit_tail(exp_sb, sums4, b, last):
            # softmax tail: sums4 rows are dense (pad rows contribute 0 via
            # the exp bias), so the batch total is one dense matmul; scale
            # split across DVE and ACT for the final batch, single DVE pass
            # otherwise.
            # partial total over column blocks 0..6 runs ~2us early (their
            # accums are long done); only cb7's single-column matmul sits on
            # the post-exp7 critical path
            sums = stat_pool.tile([128, 1], F32, tag="sums",
                                  name=f"sums{b}_{rep}")
            nc.vector.reduce_sum(sums[:], sums4[:, 0:NCB - 1], axis=AX.X)
            tot_ps = apsum_pool.tile([1, 1], F32, tag="ap", name=f"tot{b}_{rep}")
            nc.tensor.matmul(tot_ps[:], sums[:], onescol[:],
                             start=True, stop=False)
            nc.tensor.matmul(tot_ps[:], sums4[:, NCB - 1:NCB], onescol[:],
                             start=False, stop=True)
            inv = stat_pool.tile([1, 1], F32, tag="inv", name=f"iv{b}_{rep}")
            nc.vector.reciprocal(inv[:], tot_ps[:])
            inv_ps = apsum_pool.tile([128, 1], F32, tag="ap", name=f"ib{b}_{rep}")
            nc.tensor.matmul(inv_ps[:], ones128[:], inv[:], start=True, stop=True)
            inv128 = stat_pool.tile([128, 1], F32, tag="inv128",
                                    name=f"i8{b}_{rep}")
            nc.vector.tensor_copy(inv128[:], inv_ps[:])
            res = out_pool.tile([128, NCB * ST], F32, tag="res",
                                name=f"res{b}_{rep}")
            o4 = out_d[b].rearrange("(cb q u) -> q cb u", cb=NCB, q=2, u=ST)
            r4 = res[0:128:64, :].rearrange("q (cb u) -> q cb u", u=ST)
            # DVE is ~0.55ns/elem on this op vs ACT ~0.92: DVE takes 6 of 8
            # column blocks so the two halves finish together
            ncb_dve = int(os.environ.get("K_NCBDVE", "6"))
            cut = ncb_dve * ST
            if last and os.environ.get("K_TSPLIT", "1") == "1":
                nc.vector.tensor_scalar_mul(res[:, 0:cut],
                                            exp_sb[:, 0:cut], inv_ps[:])
                nc.scalar.activation(res[:, cut:], exp_sb[:, cut:],
                                     AF.Copy, scale=inv128[:])
                nc.sync.dma_start(o4[:, 0:ncb_dve], r4[:, 0:ncb_dve])
                nc.scalar.dma_start(o4[:, ncb_dve:], r4[:, ncb_dve:])
            elif last:
                nc.vector.tensor_scalar_mul(res[:], exp_sb[:], inv_ps[:])
                nc.sync.dma_start(o4, r4)
            else:
                nc.vector.tensor_scalar_mul(res[:], exp_sb[:], inv_ps[:])
                nc.gpsimd.dma_start(o4, r4)

        vq = []  # pending v-reduces, emitted a few subtiles late
        pending_tail = None

        def flush_vq(n):
            while len(vq) > n:
                w_, cbt_, rv = vq.pop(0)
                v_reduce(w_, cbt_, rv)
                if rv % 2 == 1:
                    emit_exp_cb(cbt_, vq_exp[0], vq_exp[1], rv // 2)

        cbt_cur = [None]

        for rep, b in [(rp, bb) for rp in range(reps) for bb in range(BPC)]:
            # row r lives at partition 64*(r%2) of column-block psum tile r//2
            exp_sb = out_pool.tile([128, NCB * ST], F32, tag="exp",
                                   name=f"exp{rep}_{b}")
            sums4 = stat_pool.tile([128, NCB], F32, tag="sums4",
                                   name=f"sums4_{rep}_{b}")
            vq_exp = (exp_sb, sums4)
            last_batch = (rep == reps - 1 and b == BPC - 1)
            for g in range(NG):
                split_last = last_batch and g == NG - 1 and \
                    os.environ.get("K_SPLITLAST", "0") == "1"
                if rep == 0 and b == 0 and g == 0:
                    chunks = chunks0
                elif not split_last and rep == 0 and b == 0 \
                        and 1 <= g < int(os.environ.get("K_NSINGLE", "4")):
                    # pipeline-fill groups load as singles (smaller sems,
                    # earlier consumption); saturated groups use kc-pairs
                    chunks = []
                    for kc in range(NKC):
                        c = enc_pool.tile([128, GT], BF16, tag="enc",
                                          name=f"c1s_{g}_{kc}")
                        nc.sync.dma_start(
                            c[:], encT_d[b, kc, :, g * GT:(g + 1) * GT])
                        chunks.append(c)
                elif not split_last and os.environ.get("K_QUAD", "0") == "1":
                    cq = enc_pool.tile([128, NKC, GT], BF16, tag="encq",
                                       name=f"cq{rep}_{b}_{g}")
                    nc.sync.dma_start(
                        cq[:], encT_d[b, :, :, g * GT:(g + 1) * GT].rearrange(
                            "kc p s -> p kc s"))
                    chunks = [cq[:, kc, :] for kc in range(NKC)]
                elif not split_last:
                    chunks = []
                    for pr in range(NKC // 2):
                        cp = enc_pool.tile([128, 2, GT], BF16, tag="encp",
                                           name=f"c{rep}_{b}_{g}_{pr}")
                        nc.sync.dma_start(
                            cp[:], encT_d[b, 2 * pr:2 * pr + 2, :,
                                          g * GT:(g + 1) * GT].rearrange(
                                "kc p s -> p kc s"))
                        chunks.append(cp[:, 0, :])
                        chunks.append(cp[:, 1, :])
                for si in range(NSI):
                    r = g * NSI + si
                    if split_last:
                        # final group: per-subtile 512-token loads so the last
                        # compute chain starts half a group earlier
                        chunks = []
                        t0 = g * GT + si * ST
                        for kc in range(NKC):
                            c = enc_pool.tile([128, GT], BF16, tag="enc",
                                              name=f"cl{rep}_{b}_{si}_{kc}")
                            nc.sync.dma_start(
                                c[:, 0:ST], encT_d[b, kc, :, t0:t0 + ST])
                            chunks.append(c)
                    epsums = [epsum_pool.tile([128, ST], F32, tag="ep",
                                              name=f"ep_{rep}_{b}_{r}_{i}")
                              for i in range(NHC)]
                    tanhs = []
                    for hc in range(NHC):
                        for kc in range(NKC):
                            nc.tensor.matmul(
                                epsums[hc][:],
                                w_sb[kc][:, hc * 128:(hc + 1) * 128],
                                chunks[kc][:, 0:ST] if split_last else
                                chunks[kc][:, si * ST:(si + 1) * ST],
                                start=(kc == 0), stop=(kc == NKC - 1))
                        th = tanh_pool.tile([128, ST], BF16, tag="th")
                        nc.scalar.activation(th[:], epsums[hc][:], AF.Tanh,
                                             bias=bias_sb[b][hc])
                        tanhs.append(th)
                    # fold the two h-chunks on DVE: w = t_a*(va/vb) + t_b
                    w_f = fold_pool.tile([128, ST], BF16, tag="fold")
                    nc.vector.scalar_tensor_tensor(
                        w_f[:], tanhs[0][:], r_col[:], tanhs[1][:],
                        ALU.mult, ALU.add)
                    if r % 2 == 0:
                        cbt_cur[0] = cb_pool.tile(
                            [128, ST], F32, tag="cb", name=f"cb_{rep}_{b}_{r // 2}")
                    vq.append((w_f, cbt_cur[0], r))
                    flush_vq(int(os.environ.get("K_VQ", "3")))
                if pending_tail is not None and g == 1:
                    # emit the previous batch's remaining softmax tail here so
                    # it queues behind only two groups of this batch's work
                    emit_tail(*pending_tail, last=False)
                    pending_tail = None
            # flush remaining subtiles of this batch
            flush_vq(0)
            if rep < reps - 1 or b < BPC - 1:
                pending_tail = (exp_sb, sums4, b)
                if b == BPC - 1:
                    # next rep re-enters at g==1 of its first batch
                    pass
            else:
                emit_tail(exp_sb, sums4, b, last=True)


    nc.compile()
    _CACHE[key] = nc
    return nc


def kernel(hidden, encoder_outputs, attn_w, attn_b, v):
    global LAST_RESULT
    hidden = np.asarray(hidden, dtype=np.float32)
    encoder_outputs = np.asarray(encoder_outputs, dtype=np.float32)
    attn_w = np.asarray(attn_w, dtype=np.float32)
    attn_b = np.asarray(attn_b, dtype=np.float32)
    v = np.asarray(v, dtype=np.float32)

    # host-side marshaling (tiny except the one-time layout change of enc);
    # enc/W/v are cast to bf16 so the device streams half the HBM bytes
    # (measured end-to-end rel err ~6.6e-3 vs the 2e-2 gate)
    encT = np.ascontiguousarray(
        encoder_outputs.transpose(0, 2, 1)).astype(NPBF16)           # [B, 2H, S]

    # pair h-channel p with p+128 and order each pair so the larger v is
    # second ("b"): att = sum_p va*t_a + vb*t_b is computed on-device as
    # vb * (t_a*(va/vb) + t_b) with r = va/vb <= ~1 (no range blowup).
    half = H // 2
    v0, v1 = v[:half], v[half:]
    sw = v0 > v1
    perm = np.arange(H)
    perm[:half][sw] = np.arange(half)[sw] + half
    perm[half:][sw] = np.arange(half)[sw]
    attn_w_p = attn_w[perm]
    attn_b_p = attn_b[perm]
    vperm = v[perm]
    va, vb = vperm[:half], vperm[half:]
    vb_bf = vb.astype(NPBF16)
    r = (va / np.maximum(vb_bf.astype(np.float32), 1e-20)).astype(np.float32)

    W_h = attn_w_p[:, :H]
    bias_hb = hidden[:, 0, :] @ W_h.T + attn_b_p                     # [B, H]
    wT = np.ascontiguousarray(attn_w_p[:, H:].T).reshape(
        NKC, 128, H).astype(NPBF16)                                  # [4,128,256]

    nc = _build()
    in_maps = []
    for c in range(NCORES):
        sl = slice(BPC * c, BPC * (c + 1))
        in_maps.append({
            "encT": encT[sl].reshape(BPC, NKC, 128, S),
            "wT": wT,
            "biasT": np.ascontiguousarray(bias_hb[sl]).reshape(BPC, NHC, 128, 1),
            "rT": r.reshape(128, 1),
            "vbT": vb_bf.reshape(128, 1),
        })

    trace = bool(os.environ.get("KERNEL_TRACE"))
    if trace:
        try:
            from antenv.axon_hooks import get_axon_ntff_profile_hook  # noqa: F401
        except ImportError:
            trace = False
    res = run_bass_kernel_spmd(
        nc, in_maps, core_ids=list(range(NCORES)), trace=trace)
    LAST_RESULT = res
    globals()["LAST_IN_MAPS"] = in_maps
    out = np.concatenate(
        [res.results[c]["out"].reshape(BPC, S) for c in range(NCORES)], axis=0)
    return out.reshape(B, 1, S).astype(np.float32)


if __name__ == "__main__":
    rng = np.random.default_rng(0)
    hid = rng.standard_normal((B, 1, H), dtype=np.float32)
    enc = rng.standard_normal((B, S, 2 * H), dtype=np.float32)
    aw = rng.standard_normal((H, 3 * H), dtype=np.float32) / np.sqrt(3 * H)
    ab = rng.standard_normal(H, dtype=np.float32) * 0.01
    vv = rng.random(H, dtype=np.float32)
    out = kernel(hid, enc, aw, ab, vv)
    print(out.shape, out.sum(axis=-1))

